# revision 7
# baseline (speedup 1.0000x reference)
import numpy as np
from contextlib import ExitStack

import concourse.bass as bass
import concourse.tile as tile
from concourse import mybir, bass_utils
from concourse.masks import make_identity

N, E, D, EF = 50000, 800000, 128, 64
NH, DH = 8, 16
NCORES = 8
NPC = N // NCORES           # 6250 nodes per core
W = 49                      # windows of 128 nodes per core
NPAD = W * 128              # 6272
EPS = 1e-5
EXP_BIAS = -2.7726          # exp scaled by 2^-4; cancels in ws/den ratio

F32 = mybir.dt.float32
F16 = mybir.dt.float16
I32 = mybir.dt.int32
AF = mybir.ActivationFunctionType
ALU = mybir.AluOpType
AX = mybir.AxisListType


def _preprocess(inputs):
    f32 = lambda x: np.ascontiguousarray(np.asarray(x, np.float32))
    h = f32(inputs['h'])
    ef = f32(inputs['edge_feat'])
    e_w = f32(inputs['e_w'])
    src = np.asarray(inputs['edge_index'][0], np.int64)
    dst = np.asarray(inputs['edge_index'][1], np.int64)

    order = np.argsort(dst, kind='stable')
    src_s, dst_s = src[order], dst[order]
    ew_s, ef_s = e_w[order], ef[order]

    w1cat = np.concatenate([f32(inputs['hk_W1']), f32(inputs['hv_W1'])], axis=1)
    wq1 = f32(inputs['hq_W1'])
    wq1x = np.concatenate([wq1, wq1.sum(1, keepdims=True)], 1)   # [128, 129]
    wn1 = f32(inputs['no_W1'])
    wn1a_x = np.concatenate([wn1[0:128], wn1[0:128].sum(1, keepdims=True)], 1)
    wn1h_x = np.concatenate([wn1[128:256], wn1[128:256].sum(1, keepdims=True)], 1)

    cvt = lambda x: np.ascontiguousarray(np.asarray(x, np.float16))
    shared = dict(
        w1ef=cvt(w1cat[0:EF]), w1hi=cvt(w1cat[EF:EF + 128]),
        w1hj=cvt(w1cat[EF + 128:EF + 256]),
        wq1x=cvt(wq1x), wq2=cvt(inputs['hq_W2']),
        wk2=cvt(inputs['hk_W2']), wv2=cvt(inputs['hv_W2']),
        wn1ax=cvt(wn1a_x), wn1hx=cvt(wn1h_x), wn2=cvt(inputs['no_W2']),
    )
    hT16 = h.T.astype(np.float16)        # [128, N]

    bounds = np.searchsorted(dst_s, np.arange(NCORES + 1) * NPC)
    cnt = np.zeros((NCORES, W), np.int64)
    pc = []
    for c in range(NCORES):
        lo, hi = int(bounds[c]), int(bounds[c + 1])
        dl = dst_s[lo:hi] - c * NPC
        cnt[c] = np.bincount(dl >> 7, minlength=W)
        pc.append((lo, dl))
    T = np.maximum(1, (cnt.max(axis=0) + 127) // 128)
    T = ((T + 1) // 2) * 2              # even tile count per window (macro pairs)
    base_t = np.zeros(W + 1, np.int64)
    base_t[1:] = np.cumsum(T)
    Ttot = int(base_t[-1])
    EPAD = Ttot * 128

    in_maps = []
    for c in range(NCORES):
        lo, dl = pc[c]
        ncore = len(dl)
        # flat slot of each real edge within the padded tile layout
        ws = np.zeros(W + 1, np.int64)
        ws[1:] = np.cumsum(cnt[c])
        iw = np.arange(ncore)
        wofe = np.searchsorted(ws, iw, side='right') - 1   # window of each edge
        ein = iw - ws[wofe]                                # index within window
        slot = (base_t[wofe] + (ein >> 7)) * 128 + (ein & 127)

        hdstT = np.zeros((D, EPAD), np.float16)
        hdstT[:, slot] = hT16[:, dst_s[lo:lo + ncore]]
        hsrcT = np.zeros((D, EPAD), np.float16)
        hsrcT[:, slot] = hT16[:, src_s[lo:lo + ncore]]
        eft = np.zeros((EF, EPAD), np.float16)
        eft[:, slot] = ef_s[lo:lo + ncore].T.astype(np.float16)
        eww = np.zeros(EPAD, np.float32)
        eww[slot] = ew_s[lo:lo + ncore]

        nloc = dl - (wofe << 7)                            # node idx in window
        tile_of = slot >> 7
        p_of = slot & 127
        ohen = np.zeros((128, EPAD), np.float16)
        ohen[p_of, tile_of * 128 + nloc] = 1.0
        ohne = np.zeros((128, EPAD), np.float16)
        ohne[nloc, tile_of * 128 + p_of] = 1.0

        hoT = np.zeros((D, NPAD), np.float16)
        hoT[:, :NPC] = hT16[:, c * NPC:(c + 1) * NPC]
        m = dict(shared)
        m.update(
            hT_own=hoT,
            hdstT=hdstT, hsrcT=hsrcT, efT=eft,
            ohen=ohen, ohne=ohne,
            ew_c=np.ascontiguousarray(eww.reshape(Ttot, 128).T),
        )
        in_maps.append(m)
    return in_maps, [int(x) for x in T], [int(x) for x in base_t]


def _build(T, base_t):
    Ttot = base_t[-1]
    EPAD = Ttot * 128
    Tmax = max(T)
    nc = bass.Bass(target_bir_lowering=False, debug=False)
    dt = nc.dram_tensor
    hT_own_d = dt('hT_own', [128, NPAD], F16, kind='ExternalInput')
    hdstT_d = dt('hdstT', [128, EPAD], F16, kind='ExternalInput')
    hsrcT_d = dt('hsrcT', [128, EPAD], F16, kind='ExternalInput')
    efT_d = dt('efT', [EF, EPAD], F16, kind='ExternalInput')
    ohen_d = dt('ohen', [128, EPAD], F16, kind='ExternalInput')
    ohne_d = dt('ohne', [128, EPAD], F16, kind='ExternalInput')
    ew_d = dt('ew_c', [128, Ttot], F32, kind='ExternalInput')
    wd = {}
    for nm, p, q in [('w1ef', EF, 256), ('w1hi', 128, 256), ('w1hj', 128, 256),
                     ('wq1x', 128, 129), ('wq2', 128, 128), ('wk2', 128, 128),
                     ('wv2', 128, 128), ('wn1ax', 128, 129), ('wn1hx', 128, 129),
                     ('wn2', 128, 128)]:
        wd[nm] = dt(nm, [p, q], F16, kind='ExternalInput')
    out_d = dt('out', [NPAD, D], F32, kind='ExternalOutput')

    with ExitStack() as ctx:
        tc = ctx.enter_context(tile.TileContext(nc))
        cp = ctx.enter_context(tc.tile_pool(name='consts', bufs=1))

        ident = cp.tile([128, 128], F16, name='ident')
        make_identity(nc, ident[:])
        ebias_col = cp.tile([128, 1], F32, name='ebias_col')
        nc.gpsimd.memset(ebias_col[:], float(EXP_BIAS))
        ln025_col = cp.tile([128, 1], F32, name='ln025_col')
        nc.gpsimd.memset(ln025_col[:], float(np.log(0.25)))

        wsb = {}
        for nm, dr in wd.items():
            t = cp.tile(list(dr.shape), F16, name=nm + '_s')
            nc.sync.dma_start(out=t[:], in_=dr[:])
            wsb[nm] = t

        ew_s = cp.tile([128, Ttot], F32, name='ew_s')
        nc.sync.dma_start(out=ew_s[:], in_=ew_d[:])
        hT_own = cp.tile([128, NPAD], F16, name='hT_own')
        nc.sync.dma_start(out=hT_own[:], in_=hT_own_d[:])

        wp = ctx.enter_context(tc.tile_pool(name='wp', bufs=4))
        tp = ctx.enter_context(tc.tile_pool(name='tp', bufs=6))
        pp = ctx.enter_context(tc.tile_pool(name='pp', bufs=2, space='PSUM'))
        ac = ctx.enter_context(tc.tile_pool(name='ac', bufs=2, space='PSUM'))

        def load_window(w):
            Tw, tb = T[w], base_t[w]
            s = {'w': w, 'Tw': Tw, 'tb': tb, 'NM': Tw // 2,
                 'hTw': hT_own[:, w * 128:(w + 1) * 128]}
            LW = Tw * 128
            for nm, dr, P in [('hdstT_w', hdstT_d, 128), ('hsrcT_w', hsrcT_d, 128),
                              ('eft_w', efT_d, EF), ('ohen_w', ohen_d, 128),
                              ('ohne_w', ohne_d, 128)]:
                t = wp.tile([P, Tmax * 128], F16, name=nm)
                nc.sync.dma_start(out=t[:, 0:LW], in_=dr[:, tb * 128:tb * 128 + LW])
                s[nm] = t
            # ---- q-MLP for this window (LN rstd folded out) ----
            qps_t = pp.tile([128, 512], F32, name='hdn')
            qps = qps_t[:, 0:129]
            nc.tensor.matmul(qps, s['hTw'], wsb['wq1x'][:], start=True, stop=True)
            qsb = wp.tile([128, 129], F32, name='qsb')
            nc.vector.tensor_copy(qsb[:], qps)
            qst = wp.tile([128, 7], F32, name='qst')
            # 0 ssq, 1 negmu, 2 mu2, 3 s128e, 4 var_e, 5 lnv, 6 rstd
            qscr = wp.tile([128, 128], F16, name='qscr')
            nc.scalar.activation(qscr[:], qsb[:, 0:128], AF.Square,
                                 accum_out=qst[:, 0:1])
            nc.gpsimd.tensor_scalar(qst[:, 1:2], qsb[:, 128:129], -1.0 / 128, None,
                                    op0=ALU.mult)
            nc.gpsimd.tensor_tensor(qst[:, 2:3], qst[:, 1:2], qst[:, 1:2],
                                    op=ALU.mult)
            nc.gpsimd.tensor_scalar(qst[:, 3:4], qst[:, 0:1], 1.0 / 128, float(EPS),
                                    op0=ALU.mult, op1=ALU.add)
            nc.gpsimd.tensor_tensor(qst[:, 4:5], qst[:, 3:4], qst[:, 2:3],
                                    op=ALU.subtract)
            nc.scalar.activation(qst[:, 5:6], qst[:, 4:5], AF.Ln)
            nc.scalar.activation(qst[:, 6:7], qst[:, 5:6], AF.Exp, scale=-0.5)
            qrelu = wp.tile([128, 128], F16, name='qrelu')
            nc.scalar.activation(qrelu[:], qsb[:, 0:128], AF.Relu, bias=qst[:, 1:2])
            kvps_t = pp.tile([128, 512], F16, name='kvT')
            qrT_ps = kvps_t[:, 0:128]
            nc.tensor.transpose(qrT_ps, qrelu[:], ident[:])
            qrT = wp.tile([128, 128], F16, name='qrT')
            nc.vector.tensor_copy(qrT[:], qrT_ps)
            qf_t = pp.tile([128, 512], F32, name='kv')
            qf_ps = qf_t[:, 0:128]
            nc.tensor.matmul(qf_ps, qrT[:], wsb['wq2'][:], start=True, stop=True)
            q_win = wp.tile([128, 128], F16, name='q_win')
            nc.scalar.activation(q_win[:], qf_ps, AF.Copy, scale=qst[:, 6:7])
            s['q_win'] = q_win
            s['acc'] = ac.tile([128, 136], F32, name='acc')
            return s

        def emit_macro(s, m):
            t0 = 2 * m
            gt = s['tb'] + t0
            sl = lambda g: slice((t0 + g) * 128, (t0 + g + 1) * 128)
            hdn = pp.tile([128, 512], F32, name='hdn')
            for g in range(2):
                hv = hdn[:, g * 256:(g + 1) * 256]
                nc.tensor.matmul(hv, s['eft_w'][:, sl(g)], wsb['w1ef'][:],
                                 start=True, stop=False)
                nc.tensor.matmul(hv, s['hdstT_w'][:, sl(g)], wsb['w1hi'][:],
                                 start=False, stop=False)
                nc.tensor.matmul(hv, s['hsrcT_w'][:, sl(g)], wsb['w1hj'][:],
                                 start=False, stop=True)
            # stats (storage k0 v0 k1 v1): 0:4 sums, 4:8 negmu, 8:12 mu2,
            # 12:16 ssq, 16:20 s128e, 20:24 var_e, 24:28 lnv, 28:32 rstd
            # reduces/Square/relu read hdn PSUM directly (no x_sb copy);
            # wide elementwise stays OFF gpsimd (~2us/op there vs ~0.2us DVE)
            st = tp.tile([128, 32], F32, name='st')
            x4 = hdn[:].rearrange('p (q c) -> p q c', c=128)
            nc.vector.reduce_sum(out=st[:, 0:4], in_=x4, axis=AX.X)
            sq = tp.tile([128, 512], F16, name='sq')
            nc.scalar.activation(sq[:], hdn[:], AF.Square)
            nc.vector.reduce_sum(
                out=st[:, 12:16],
                in_=sq[:].rearrange('p (q c) -> p q c', c=128), axis=AX.X)
            nc.gpsimd.tensor_scalar(st[:, 4:8], st[:, 0:4], -1.0 / 128, None,
                                    op0=ALU.mult)
            nc.gpsimd.tensor_tensor(st[:, 8:12], st[:, 4:8], st[:, 4:8],
                                    op=ALU.mult)
            nc.gpsimd.tensor_scalar(st[:, 16:20], st[:, 12:16], 1.0 / 128,
                                    float(EPS), op0=ALU.mult, op1=ALU.add)
            nc.gpsimd.tensor_tensor(st[:, 20:24], st[:, 16:20], st[:, 8:12],
                                    op=ALU.subtract)
            nc.scalar.activation(st[:, 24:28], st[:, 20:24], AF.Ln)
            nc.scalar.activation(st[:, 28:32], st[:, 24:28], AF.Exp, scale=-0.5)
            # relu(x - mu) in f16 (rstd folded out)
            relu1 = tp.tile([128, 512], F16, name='relu1')
            for g in range(4):
                nc.vector.tensor_scalar(
                    relu1[:, g * 128:(g + 1) * 128],
                    hdn[:, g * 128:(g + 1) * 128],
                    st[:, 4 + g:5 + g], 0.0, op0=ALU.add, op1=ALU.max)
            kvT_ps = pp.tile([128, 512], F16, name='kvT')
            for g in range(4):
                nc.tensor.transpose(kvT_ps[:, g * 128:(g + 1) * 128],
                                    relu1[:, g * 128:(g + 1) * 128], ident[:])
            kvT = tp.tile([128, 512], F16, name='kvT_sb')
            nc.scalar.activation(kvT[:], kvT_ps[:], AF.Copy)
            kv = pp.tile([128, 512], F32, name='kv')
            for g, wnm in enumerate(['wk2', 'wv2', 'wk2', 'wv2']):
                nc.tensor.matmul(kv[:, g * 128:(g + 1) * 128],
                                 kvT[:, g * 128:(g + 1) * 128], wsb[wnm][:],
                                 start=True, stop=True)
            kv3 = kv[:].rearrange('p (t c) -> p t c', c=256)
            # qe = q[dst] via one-hot matmuls, into the (now dead) hdn bank
            qe_ps = hdn[:, 0:256]
            for g in range(2):
                nc.tensor.matmul(qe_ps[:, g * 128:(g + 1) * 128],
                                 s['ohne_w'][:, sl(g)], s['q_win'][:],
                                 start=True, stop=True)
            qe_sb = tp.tile([128, 256], F16, name='qe_sb')
            nc.scalar.activation(qe_sb[:], qe_ps, AF.Copy)
            # logits
            qk = tp.tile([128, 256], F16, name='qk')
            nc.vector.tensor_tensor(
                qk[:].rearrange('p (t c) -> p t c', c=128),
                qe_sb[:].rearrange('p (t c) -> p t c', c=128),
                kv3[:, :, 0:128], op=ALU.mult)
            lg = tp.tile([128, 16], F32, name='lg')
            nc.vector.reduce_sum(
                out=lg[:], in_=qk[:].rearrange('p (th d) -> p th d', d=16),
                axis=AX.X)
            rstd2 = st[:, 28:32].rearrange('p (t g) -> p t g', g=2)
            lgs = tp.tile([128, 16], F32, name='lgs')
            nc.gpsimd.tensor_tensor(
                lgs[:].rearrange('p (t h) -> p t h', h=NH),
                lg[:].rearrange('p (t h) -> p t h', h=NH),
                rstd2[:, :, 0:1].to_broadcast([128, 2, NH]), op=ALU.mult)
            exw = tp.tile([128, 16], F32, name='exw')
            nc.scalar.activation(exw[:], lgs[:], AF.Exp, scale=0.25,
                                 bias=ebias_col[:])
            wr = tp.tile([128, 18], F32, name='wr')
            nc.gpsimd.tensor_tensor(
                wr[:, 16:18].rearrange('p (t o) -> p t o', o=1),
                ew_s[:, gt:gt + 2].rearrange('p (t o) -> p t o', o=1),
                rstd2[:, :, 1:2], op=ALU.mult)
            nc.gpsimd.tensor_tensor(
                wr[:, 0:16].rearrange('p (t h) -> p t h', h=NH),
                exw[:].rearrange('p (t h) -> p t h', h=NH),
                wr[:, 16:18].unsqueeze(2).to_broadcast([128, 2, NH]),
                op=ALU.mult)
            # X = [v_raw * (ex*ew*rstd_v) | ex]
            X = tp.tile([128, 272], F16, name='X')
            X3 = X[:].rearrange('p (t c) -> p t c', c=136)
            nc.vector.tensor_tensor(
                X3[:, :, 0:128].rearrange('p t (h d) -> p t h d', d=DH),
                kv3[:, :, 128:256].rearrange('p t (h d) -> p t h d', d=DH),
                wr[:, 0:16].rearrange('p (t h) -> p t h', h=NH).unsqueeze(
                    3).to_broadcast([128, 2, NH, DH]), op=ALU.mult)
            nc.gpsimd.tensor_copy(
                X3[:, :, 128:136],
                exw[:].rearrange('p (t h) -> p t h', h=NH))
            for i in range(2):
                nc.tensor.matmul(s['acc'][:], s['ohen_w'][:, sl(i)],
                                 X[:, i * 136:(i + 1) * 136],
                                 start=(m == 0 and i == 0),
                                 stop=(m == s['NM'] - 1 and i == 1))

        def emit_tail(s):
            w = s['w']
            acc = s['acc']
            den = wp.tile([128, 8], F32, name='den')
            nc.vector.tensor_scalar(den[:], acc[:, 128:136], 1e-30, None,
                                    op0=ALU.max)
            rden = wp.tile([128, 8], F32, name='rden')
            nc.vector.reciprocal(rden[:], den[:])
            attn = wp.tile([128, 128], F16, name='attn')
            nc.vector.tensor_tensor(
                attn[:].rearrange('p (h d) -> p h d', d=DH),
                acc[:, 0:128].rearrange('p (h d) -> p h d', d=DH),
                rden[:].unsqueeze(2).to_broadcast([128, NH, DH]), op=ALU.mult)
            Ah = pp.tile([128, 512], F16, name='kvT')
            attnT_ps = Ah[:, 0:128]
            nc.tensor.transpose(attnT_ps, attn[:], ident[:])
            attnT = wp.tile([128, 128], F16, name='attnT')
            nc.vector.tensor_copy(attnT[:], attnT_ps)
            An = pp.tile([128, 512], F32, name='hdn')
            no_ps = An[:, 0:129]
            nc.tensor.matmul(no_ps, attnT[:], wsb['wn1ax'][:], start=True,
                             stop=False)
            nc.tensor.matmul(no_ps, s['hTw'], wsb['wn1hx'][:], start=False,
                             stop=True)
            nosb = wp.tile([128, 129], F32, name='nosb')
            nc.vector.tensor_copy(nosb[:], no_ps)
            scr2 = wp.tile([128, 128], F16, name='scr2')
            stn = wp.tile([128, 7], F32, name='stn')
            nc.scalar.activation(scr2[:], nosb[:, 0:128], AF.Square,
                                 accum_out=stn[:, 0:1])
            nc.gpsimd.tensor_scalar(stn[:, 1:2], nosb[:, 128:129], -1.0 / 128,
                                    None, op0=ALU.mult)
            nc.gpsimd.tensor_tensor(stn[:, 2:3], stn[:, 1:2], stn[:, 1:2],
                                    op=ALU.mult)
            nc.gpsimd.tensor_scalar(stn[:, 3:4], stn[:, 0:1], 1.0 / 128,
                                    float(EPS), op0=ALU.mult, op1=ALU.add)
            nc.gpsimd.tensor_tensor(stn[:, 4:5], stn[:, 3:4], stn[:, 2:3],
                                    op=ALU.subtract)
            nc.scalar.activation(stn[:, 5:6], stn[:, 4:5], AF.Ln)
            nc.scalar.activation(stn[:, 6:7], stn[:, 5:6], AF.Exp, scale=-0.5)
            norelu = wp.tile([128, 128], F16, name='norelu')
            nc.scalar.activation(norelu[:], no_ps[:, 0:128], AF.Relu,
                                 bias=stn[:, 1:2])
            norT_ps = Ah[:, 128:256]
            nc.tensor.transpose(norT_ps, norelu[:], ident[:])
            norT = wp.tile([128, 128], F16, name='norT')
            nc.vector.tensor_copy(norT[:], norT_ps)
            out_ps = pp.tile([128, 512], F32, name='kv')
            nc.tensor.matmul(out_ps[:, 0:128], norT[:], wsb['wn2'][:], start=True,
                             stop=True)
            out_sb = wp.tile([128, 128], F32, name='out_sb')
            nc.scalar.activation(out_sb[:], out_ps[:, 0:128], AF.Copy,
                                 scale=stn[:, 6:7])
            nc.sync.dma_start(out=out_d[w * 128:(w + 1) * 128, :], in_=out_sb[:])

        worder = sorted(range(W), key=lambda w: (T[w], w))
        loaded = [load_window(worder[j]) for j in (0, 1) if j < W]
        for i0 in range(0, W, 2):
            ws = loaded
            loaded = [load_window(worder[j]) for j in (i0 + 2, i0 + 3) if j < W]
            for m in range(max(s['NM'] for s in ws)):
                for s in ws:
                    if m < s['NM']:
                        emit_macro(s, m)
            for s in ws:
                emit_tail(s)
    return nc


def kernel(_trace=False, **inputs):
    import bass_rust
    in_maps, T, base_t = _preprocess(inputs)
    nc = _build(T, base_t)
    bass_rust.generate_event_semaphores(nc)
    res = bass_utils.run_bass_kernel_spmd(nc, in_maps, core_ids=list(range(NCORES)),
                                          trace=_trace)
    out = np.concatenate(
        [np.asarray(res.results[c]['out'])[:NPC] for c in range(NCORES)], axis=0)
    if _trace:
        return out.astype(np.float32), res
    return out.astype(np.float32)



# revision 17
# speedup vs baseline: 1.0221x; 1.0221x over previous
import numpy as np
from contextlib import ExitStack

import concourse.bass as bass
import concourse.tile as tile
from concourse import mybir, bass_utils
from concourse.masks import make_identity

N, E, D, EF = 50000, 800000, 128, 64
NH, DH = 8, 16
NCORES = 8
NPC = N // NCORES           # 6250 nodes per core
W = 49                      # windows of 128 nodes per core
NPAD = W * 128              # 6272
EPS = 1e-5
EXP_BIAS = -2.7726          # exp scaled by 2^-4; cancels in ws/den ratio

F32 = mybir.dt.float32
F16 = mybir.dt.float16
I32 = mybir.dt.int32
AF = mybir.ActivationFunctionType
ALU = mybir.AluOpType
AX = mybir.AxisListType


def _preprocess(inputs):
    f32 = lambda x: np.ascontiguousarray(np.asarray(x, np.float32))
    h = f32(inputs['h'])
    ef = f32(inputs['edge_feat'])
    e_w = f32(inputs['e_w'])
    src = np.asarray(inputs['edge_index'][0], np.int64)
    dst = np.asarray(inputs['edge_index'][1], np.int64)

    order = np.argsort(dst, kind='stable')
    src_s, dst_s = src[order], dst[order]
    ew_s, ef_s = e_w[order], ef[order]

    w1cat = np.concatenate([f32(inputs['hk_W1']), f32(inputs['hv_W1'])], axis=1)
    wq1 = f32(inputs['hq_W1'])
    wq1x = np.concatenate([wq1, wq1.sum(1, keepdims=True)], 1)   # [128, 129]
    wn1 = f32(inputs['no_W1'])
    wn1a_x = np.concatenate([wn1[0:128], wn1[0:128].sum(1, keepdims=True)], 1)
    wn1h_x = np.concatenate([wn1[128:256], wn1[128:256].sum(1, keepdims=True)], 1)

    cvt = lambda x: np.ascontiguousarray(np.asarray(x, np.float16))
    shared = dict(
        w1ef=cvt(w1cat[0:EF]), w1hi=cvt(w1cat[EF:EF + 128]),
        w1hj=cvt(w1cat[EF + 128:EF + 256]),
        wq1x=cvt(wq1x), wq2=cvt(inputs['hq_W2']),
        wk2=cvt(inputs['hk_W2']), wv2=cvt(inputs['hv_W2']),
        wn1ax=cvt(wn1a_x), wn1hx=cvt(wn1h_x), wn2=cvt(inputs['no_W2']),
    )
    hT16 = h.T.astype(np.float16)        # [128, N]

    bounds = np.searchsorted(dst_s, np.arange(NCORES + 1) * NPC)
    cnt = np.zeros((NCORES, W), np.int64)
    pc = []
    for c in range(NCORES):
        lo, hi = int(bounds[c]), int(bounds[c + 1])
        dl = dst_s[lo:hi] - c * NPC
        cnt[c] = np.bincount(dl >> 7, minlength=W)
        pc.append((lo, dl))
    T = np.maximum(1, (cnt.max(axis=0) + 127) // 128)
    T = ((T + 1) // 2) * 2              # even tile count per window (macro pairs)
    base_t = np.zeros(W + 1, np.int64)
    base_t[1:] = np.cumsum(T)
    Ttot = int(base_t[-1])
    EPAD = Ttot * 128

    in_maps = []
    for c in range(NCORES):
        lo, dl = pc[c]
        ncore = len(dl)
        # flat slot of each real edge within the padded tile layout
        ws = np.zeros(W + 1, np.int64)
        ws[1:] = np.cumsum(cnt[c])
        iw = np.arange(ncore)
        wofe = np.searchsorted(ws, iw, side='right') - 1   # window of each edge
        ein = iw - ws[wofe]                                # index within window
        slot = (base_t[wofe] + (ein >> 7)) * 128 + (ein & 127)

        hdstT = np.zeros((D, EPAD), np.float16)
        hdstT[:, slot] = hT16[:, dst_s[lo:lo + ncore]]
        hsrcT = np.zeros((D, EPAD), np.float16)
        hsrcT[:, slot] = hT16[:, src_s[lo:lo + ncore]]
        eft = np.zeros((EF, EPAD), np.float16)
        eft[:, slot] = ef_s[lo:lo + ncore].T.astype(np.float16)
        eww = np.zeros(EPAD, np.float32)
        eww[slot] = ew_s[lo:lo + ncore]

        nloc = dl - (wofe << 7)                            # node idx in window
        tile_of = slot >> 7
        p_of = slot & 127
        ohen = np.zeros((128, EPAD), np.float16)
        ohen[p_of, tile_of * 128 + nloc] = 1.0
        ohne = np.zeros((128, EPAD), np.float16)
        ohne[nloc, tile_of * 128 + p_of] = 1.0

        hoT = np.zeros((D, NPAD), np.float16)
        hoT[:, :NPC] = hT16[:, c * NPC:(c + 1) * NPC]
        m = dict(shared)
        m.update(
            hT_own=hoT,
            hdstT=hdstT, hsrcT=hsrcT, efT=eft,
            ohen=ohen, ohne=ohne,
            ew_c=np.ascontiguousarray(eww.reshape(Ttot, 128).T),
        )
        in_maps.append(m)
    return in_maps, [int(x) for x in T], [int(x) for x in base_t]


def _build(T, base_t):
    Ttot = base_t[-1]
    EPAD = Ttot * 128
    Tmax = max(T)
    nc = bass.Bass(target_bir_lowering=False, debug=False)
    dt = nc.dram_tensor
    hT_own_d = dt('hT_own', [128, NPAD], F16, kind='ExternalInput')
    hdstT_d = dt('hdstT', [128, EPAD], F16, kind='ExternalInput')
    hsrcT_d = dt('hsrcT', [128, EPAD], F16, kind='ExternalInput')
    efT_d = dt('efT', [EF, EPAD], F16, kind='ExternalInput')
    ohen_d = dt('ohen', [128, EPAD], F16, kind='ExternalInput')
    ohne_d = dt('ohne', [128, EPAD], F16, kind='ExternalInput')
    ew_d = dt('ew_c', [128, Ttot], F32, kind='ExternalInput')
    wd = {}
    for nm, p, q in [('w1ef', EF, 256), ('w1hi', 128, 256), ('w1hj', 128, 256),
                     ('wq1x', 128, 129), ('wq2', 128, 128), ('wk2', 128, 128),
                     ('wv2', 128, 128), ('wn1ax', 128, 129), ('wn1hx', 128, 129),
                     ('wn2', 128, 128)]:
        wd[nm] = dt(nm, [p, q], F16, kind='ExternalInput')
    out_d = dt('out', [NPAD, D], F32, kind='ExternalOutput')

    with ExitStack() as ctx:
        tc = ctx.enter_context(tile.TileContext(nc))
        cp = ctx.enter_context(tc.tile_pool(name='consts', bufs=1))

        ident = cp.tile([128, 128], F16, name='ident')
        make_identity(nc, ident[:])
        ebias_col = cp.tile([128, 1], F32, name='ebias_col')
        nc.gpsimd.memset(ebias_col[:], float(EXP_BIAS))
        ln025_col = cp.tile([128, 1], F32, name='ln025_col')
        nc.gpsimd.memset(ln025_col[:], float(np.log(0.25)))

        wsb = {}
        for nm, dr in wd.items():
            t = cp.tile(list(dr.shape), F16, name=nm + '_s')
            nc.sync.dma_start(out=t[:], in_=dr[:])
            wsb[nm] = t

        ew_s = cp.tile([128, Ttot], F32, name='ew_s')
        nc.sync.dma_start(out=ew_s[:], in_=ew_d[:])
        hT_own = cp.tile([128, NPAD], F16, name='hT_own')
        nc.sync.dma_start(out=hT_own[:], in_=hT_own_d[:])

        wp = ctx.enter_context(tc.tile_pool(name='wp', bufs=4))
        tp = ctx.enter_context(tc.tile_pool(name='tp', bufs=6))
        pp = ctx.enter_context(tc.tile_pool(name='pp', bufs=2, space='PSUM'))
        ph = ctx.enter_context(tc.tile_pool(name='ph', bufs=2, space='PSUM'))
        ac = ctx.enter_context(tc.tile_pool(name='ac', bufs=2, space='PSUM'))

        def load_window(w):
            Tw, tb = T[w], base_t[w]
            s = {'w': w, 'Tw': Tw, 'tb': tb, 'NM': Tw // 2,
                 'hTw': hT_own[:, w * 128:(w + 1) * 128]}
            LW = Tw * 128
            for nm, dr, P in [('hdstT_w', hdstT_d, 128), ('hsrcT_w', hsrcT_d, 128),
                              ('eft_w', efT_d, EF), ('ohen_w', ohen_d, 128),
                              ('ohne_w', ohne_d, 128)]:
                t = wp.tile([P, Tmax * 128], F16, name=nm)
                nc.sync.dma_start(out=t[:, 0:LW], in_=dr[:, tb * 128:tb * 128 + LW])
                s[nm] = t
            # ---- q-MLP for this window (LN rstd folded out) ----
            qps_t = ph.tile([128, 512], F32, name='hdn')
            qps = qps_t[:, 0:129]
            nc.tensor.matmul(qps, s['hTw'], wsb['wq1x'][:], start=True, stop=True)
            qsb = wp.tile([128, 129], F32, name='qsb')
            nc.vector.tensor_copy(qsb[:], qps)
            qst = wp.tile([128, 7], F32, name='qst')
            # 0 ssq, 1 negmu, 2 mu2, 3 s128e, 4 var_e, 5 lnv, 6 rstd
            qscr = wp.tile([128, 128], F16, name='qscr')
            nc.scalar.activation(qscr[:], qsb[:, 0:128], AF.Square,
                                 accum_out=qst[:, 0:1])
            nc.gpsimd.tensor_scalar(qst[:, 1:2], qsb[:, 128:129], -1.0 / 128, None,
                                    op0=ALU.mult)
            nc.gpsimd.tensor_tensor(qst[:, 2:3], qst[:, 1:2], qst[:, 1:2],
                                    op=ALU.mult)
            nc.gpsimd.tensor_scalar(qst[:, 3:4], qst[:, 0:1], 1.0 / 128, float(EPS),
                                    op0=ALU.mult, op1=ALU.add)
            nc.gpsimd.tensor_tensor(qst[:, 4:5], qst[:, 3:4], qst[:, 2:3],
                                    op=ALU.subtract)
            nc.scalar.activation(qst[:, 5:6], qst[:, 4:5], AF.Ln)
            nc.scalar.activation(qst[:, 6:7], qst[:, 5:6], AF.Exp, scale=-0.5)
            qrelu = wp.tile([128, 128], F16, name='qrelu')
            nc.scalar.activation(qrelu[:], qsb[:, 0:128], AF.Relu, bias=qst[:, 1:2])
            kvps_t = pp.tile([128, 512], F16, name='kvT')
            qrT_ps = kvps_t[:, 0:128]
            nc.tensor.transpose(qrT_ps, qrelu[:], ident[:])
            qrT = wp.tile([128, 128], F16, name='qrT')
            nc.vector.tensor_copy(qrT[:], qrT_ps)
            qf_t = pp.tile([128, 512], F32, name='kv')
            qf_ps = qf_t[:, 0:128]
            nc.tensor.matmul(qf_ps, qrT[:], wsb['wq2'][:], start=True, stop=True)
            q_win = wp.tile([128, 128], F16, name='q_win')
            nc.scalar.activation(q_win[:], qf_ps, AF.Copy, scale=qst[:, 6:7])
            s['q_win'] = q_win
            s['acc'] = ac.tile([128, 136], F32, name='acc')
            return s

        def emit_macro(s, m):
            t0 = 2 * m
            gt = s['tb'] + t0
            sl = lambda g: slice((t0 + g) * 128, (t0 + g + 1) * 128)
            hdn = ph.tile([128, 512], F32, name='hdn')
            for g in range(2):
                hv = hdn[:, g * 256:(g + 1) * 256]
                nc.tensor.matmul(hv, s['eft_w'][:, sl(g)], wsb['w1ef'][:],
                                 start=True, stop=False)
                nc.tensor.matmul(hv, s['hdstT_w'][:, sl(g)], wsb['w1hi'][:],
                                 start=False, stop=False)
                nc.tensor.matmul(hv, s['hsrcT_w'][:, sl(g)], wsb['w1hj'][:],
                                 start=False, stop=True)
            # stats (storage k0 v0 k1 v1): 0:4 sums, 4:8 negmu, 8:12 mu2,
            # 12:16 ssq, 16:20 s128e, 20:24 var_e, 24:28 lnv, 28:32 rstd
            # reduces/Square/relu read hdn PSUM directly (no x_sb copy);
            # wide elementwise stays OFF gpsimd (~2us/op there vs ~0.2us DVE)
            st = tp.tile([128, 32], F32, name='st')
            x4 = hdn[:].rearrange('p (q c) -> p q c', c=128)
            nc.vector.reduce_sum(out=st[:, 0:4], in_=x4, axis=AX.X)
            sq = tp.tile([128, 512], F16, name='sq')
            nc.scalar.activation(sq[:], hdn[:], AF.Square)
            nc.vector.reduce_sum(
                out=st[:, 12:16],
                in_=sq[:].rearrange('p (q c) -> p q c', c=128), axis=AX.X)
            nc.gpsimd.tensor_scalar(st[:, 4:8], st[:, 0:4], -1.0 / 128, None,
                                    op0=ALU.mult)
            nc.gpsimd.tensor_tensor(st[:, 8:12], st[:, 4:8], st[:, 4:8],
                                    op=ALU.mult)
            nc.gpsimd.tensor_scalar(st[:, 16:20], st[:, 12:16], 1.0 / 128,
                                    float(EPS), op0=ALU.mult, op1=ALU.add)
            nc.gpsimd.tensor_tensor(st[:, 20:24], st[:, 16:20], st[:, 8:12],
                                    op=ALU.subtract)
            nc.scalar.activation(st[:, 24:28], st[:, 20:24], AF.Ln)
            nc.scalar.activation(st[:, 28:32], st[:, 24:28], AF.Exp, scale=-0.5)
            # relu(x - mu) in f16 (rstd folded out)
            relu1 = tp.tile([128, 512], F16, name='relu1')
            for g in range(4):
                if g % 2 == 0:
                    nc.vector.tensor_scalar(
                        relu1[:, g * 128:(g + 1) * 128],
                        hdn[:, g * 128:(g + 1) * 128],
                        st[:, 4 + g:5 + g], 0.0, op0=ALU.add, op1=ALU.max)
                else:
                    nc.scalar.activation(
                        relu1[:, g * 128:(g + 1) * 128],
                        hdn[:, g * 128:(g + 1) * 128],
                        AF.Relu, bias=st[:, 4 + g:5 + g])
            kvq_t = pp.tile([128, 1024], F16, name='kvT')
            kvT_ps = kvq_t[:, 0:512]
            for g in range(4):
                nc.tensor.transpose(kvT_ps[:, g * 128:(g + 1) * 128],
                                    relu1[:, g * 128:(g + 1) * 128], ident[:])
            kvT = tp.tile([128, 512], F16, name='kvT_sb')
            nc.scalar.activation(kvT[:], kvT_ps[:], AF.Copy)
            kv = pp.tile([128, 512], F32, name='kv')
            for g, wnm in enumerate(['wk2', 'wv2', 'wk2', 'wv2']):
                nc.tensor.matmul(kv[:, g * 128:(g + 1) * 128],
                                 kvT[:, g * 128:(g + 1) * 128], wsb[wnm][:],
                                 start=True, stop=True)
            kv3 = kv[:].rearrange('p (t c) -> p t c', c=256)
            # qe = q[dst] via one-hot matmuls (shares the kvT bank; frees hdn early)
            qe_ps = kvq_t[:, 512:1024].bitcast(F32)
            for g in range(2):
                nc.tensor.matmul(qe_ps[:, g * 128:(g + 1) * 128],
                                 s['ohne_w'][:, sl(g)], s['q_win'][:],
                                 start=True, stop=True)
            qe_sb = tp.tile([128, 256], F16, name='qe_sb')
            nc.scalar.activation(qe_sb[:], qe_ps, AF.Copy)
            # logits
            qk = tp.tile([128, 256], F16, name='qk')
            nc.vector.tensor_tensor(
                qk[:].rearrange('p (t c) -> p t c', c=128),
                qe_sb[:].rearrange('p (t c) -> p t c', c=128),
                kv3[:, :, 0:128], op=ALU.mult)
            lg = tp.tile([128, 16], F32, name='lg')
            nc.vector.reduce_sum(
                out=lg[:], in_=qk[:].rearrange('p (th d) -> p th d', d=16),
                axis=AX.X)
            rstd2 = st[:, 28:32].rearrange('p (t g) -> p t g', g=2)
            lgs = tp.tile([128, 16], F32, name='lgs')
            nc.gpsimd.tensor_tensor(
                lgs[:].rearrange('p (t h) -> p t h', h=NH),
                lg[:].rearrange('p (t h) -> p t h', h=NH),
                rstd2[:, :, 0:1].to_broadcast([128, 2, NH]), op=ALU.mult)
            exw = tp.tile([128, 16], F32, name='exw')
            nc.scalar.activation(exw[:], lgs[:], AF.Exp, scale=0.25,
                                 bias=ebias_col[:])
            wr = tp.tile([128, 18], F32, name='wr')
            nc.gpsimd.tensor_tensor(
                wr[:, 16:18].rearrange('p (t o) -> p t o', o=1),
                ew_s[:, gt:gt + 2].rearrange('p (t o) -> p t o', o=1),
                rstd2[:, :, 1:2], op=ALU.mult)
            nc.gpsimd.tensor_tensor(
                wr[:, 0:16].rearrange('p (t h) -> p t h', h=NH),
                exw[:].rearrange('p (t h) -> p t h', h=NH),
                wr[:, 16:18].unsqueeze(2).to_broadcast([128, 2, NH]),
                op=ALU.mult)
            # X = [v_raw * (ex*ew*rstd_v) | ex]
            X = tp.tile([128, 272], F16, name='X')
            X3 = X[:].rearrange('p (t c) -> p t c', c=136)
            nc.vector.tensor_tensor(
                X3[:, :, 0:128].rearrange('p t (h d) -> p t h d', d=DH),
                kv3[:, :, 128:256].rearrange('p t (h d) -> p t h d', d=DH),
                wr[:, 0:16].rearrange('p (t h) -> p t h', h=NH).unsqueeze(
                    3).to_broadcast([128, 2, NH, DH]), op=ALU.mult)
            nc.gpsimd.tensor_copy(
                X3[:, :, 128:136],
                exw[:].rearrange('p (t h) -> p t h', h=NH))
            for i in range(2):
                nc.tensor.matmul(s['acc'][:], s['ohen_w'][:, sl(i)],
                                 X[:, i * 136:(i + 1) * 136],
                                 start=(m == 0 and i == 0),
                                 stop=(m == s['NM'] - 1 and i == 1))

        def emit_tail(s):
            w = s['w']
            acc = s['acc']
            den = wp.tile([128, 8], F32, name='den')
            nc.vector.tensor_scalar(den[:], acc[:, 128:136], 1e-30, None,
                                    op0=ALU.max)
            rden = wp.tile([128, 8], F32, name='rden')
            nc.vector.reciprocal(rden[:], den[:])
            attn = wp.tile([128, 128], F16, name='attn')
            nc.vector.tensor_tensor(
                attn[:].rearrange('p (h d) -> p h d', d=DH),
                acc[:, 0:128].rearrange('p (h d) -> p h d', d=DH),
                rden[:].unsqueeze(2).to_broadcast([128, NH, DH]), op=ALU.mult)
            Ah = pp.tile([128, 1024], F16, name='kvT')
            attnT_ps = Ah[:, 0:128]
            nc.tensor.transpose(attnT_ps, attn[:], ident[:])
            attnT = wp.tile([128, 128], F16, name='attnT')
            nc.vector.tensor_copy(attnT[:], attnT_ps)
            An = ph.tile([128, 512], F32, name='hdn')
            no_ps = An[:, 0:129]
            nc.tensor.matmul(no_ps, attnT[:], wsb['wn1ax'][:], start=True,
                             stop=False)
            nc.tensor.matmul(no_ps, s['hTw'], wsb['wn1hx'][:], start=False,
                             stop=True)
            nosb = wp.tile([128, 129], F32, name='nosb')
            nc.vector.tensor_copy(nosb[:], no_ps)
            scr2 = wp.tile([128, 128], F16, name='scr2')
            stn = wp.tile([128, 7], F32, name='stn')
            nc.scalar.activation(scr2[:], nosb[:, 0:128], AF.Square,
                                 accum_out=stn[:, 0:1])
            nc.gpsimd.tensor_scalar(stn[:, 1:2], nosb[:, 128:129], -1.0 / 128,
                                    None, op0=ALU.mult)
            nc.gpsimd.tensor_tensor(stn[:, 2:3], stn[:, 1:2], stn[:, 1:2],
                                    op=ALU.mult)
            nc.gpsimd.tensor_scalar(stn[:, 3:4], stn[:, 0:1], 1.0 / 128,
                                    float(EPS), op0=ALU.mult, op1=ALU.add)
            nc.gpsimd.tensor_tensor(stn[:, 4:5], stn[:, 3:4], stn[:, 2:3],
                                    op=ALU.subtract)
            nc.scalar.activation(stn[:, 5:6], stn[:, 4:5], AF.Ln)
            nc.scalar.activation(stn[:, 6:7], stn[:, 5:6], AF.Exp, scale=-0.5)
            norelu = wp.tile([128, 128], F16, name='norelu')
            nc.scalar.activation(norelu[:], no_ps[:, 0:128], AF.Relu,
                                 bias=stn[:, 1:2])
            norT_ps = Ah[:, 128:256]
            nc.tensor.transpose(norT_ps, norelu[:], ident[:])
            norT = wp.tile([128, 128], F16, name='norT')
            nc.vector.tensor_copy(norT[:], norT_ps)
            out_ps = pp.tile([128, 512], F32, name='kv')
            nc.tensor.matmul(out_ps[:, 0:128], norT[:], wsb['wn2'][:], start=True,
                             stop=True)
            out_sb = wp.tile([128, 128], F32, name='out_sb')
            nc.scalar.activation(out_sb[:], out_ps[:, 0:128], AF.Copy,
                                 scale=stn[:, 6:7])
            nc.sync.dma_start(out=out_d[w * 128:(w + 1) * 128, :], in_=out_sb[:])

        worder = sorted(range(W), key=lambda w: (T[w], w))
        loaded = [load_window(worder[j]) for j in (0, 1) if j < W]
        for i0 in range(0, W, 2):
            ws = loaded
            loaded = [load_window(worder[j]) for j in (i0 + 2, i0 + 3) if j < W]
            for m in range(max(s['NM'] for s in ws)):
                for s in ws:
                    if m < s['NM']:
                        emit_macro(s, m)
            for s in ws:
                emit_tail(s)
    return nc


def kernel(_trace=False, **inputs):
    import bass_rust
    in_maps, T, base_t = _preprocess(inputs)
    nc = _build(T, base_t)
    bass_rust.generate_event_semaphores(nc)
    res = bass_utils.run_bass_kernel_spmd(nc, in_maps, core_ids=list(range(NCORES)),
                                          trace=_trace)
    out = np.concatenate(
        [np.asarray(res.results[c]['out'])[:NPC] for c in range(NCORES)], axis=0)
    if _trace:
        return out.astype(np.float32), res
    return out.astype(np.float32)



# revision 22
# speedup vs baseline: 35.3110x; 34.5470x over previous
import numpy as np
from contextlib import ExitStack

import concourse.bass as bass
import concourse.tile as tile
from concourse import mybir, bass_utils
from concourse.masks import make_identity

N, E, D, EF = 50000, 800000, 128, 64
NH, DH = 8, 16
NCORES = 8
NPC = N // NCORES           # 6250 nodes per core
W = 49                      # windows of 128 nodes per core
NPAD = W * 128              # 6272
EPS = 1e-5
EXP_BIAS = -2.7726          # exp scaled by 2^-4; cancels in ws/den ratio

F32 = mybir.dt.float32
F16 = mybir.dt.float16
I32 = mybir.dt.int32
AF = mybir.ActivationFunctionType
ALU = mybir.AluOpType
AX = mybir.AxisListType


def _preprocess(inputs):
    f32 = lambda x: np.ascontiguousarray(np.asarray(x, np.float32))
    h = f32(inputs['h'])
    ef = f32(inputs['edge_feat'])
    e_w = f32(inputs['e_w'])
    src = np.asarray(inputs['edge_index'][0], np.int64)
    dst = np.asarray(inputs['edge_index'][1], np.int64)

    order = np.argsort(dst, kind='stable')
    src_s, dst_s = src[order], dst[order]
    ew_s, ef_s = e_w[order], ef[order]

    w1cat = np.concatenate([f32(inputs['hk_W1']), f32(inputs['hv_W1'])], axis=1)
    wq1 = f32(inputs['hq_W1'])
    wq1x = np.concatenate([wq1, wq1.sum(1, keepdims=True)], 1)   # [128, 129]
    wn1 = f32(inputs['no_W1'])
    wn1a_x = np.concatenate([wn1[0:128], wn1[0:128].sum(1, keepdims=True)], 1)
    wn1h_x = np.concatenate([wn1[128:256], wn1[128:256].sum(1, keepdims=True)], 1)

    cvt = lambda x: np.ascontiguousarray(np.asarray(x, np.float16))
    shared = dict(
        w1ef=cvt(w1cat[0:EF]), w1hi=cvt(w1cat[EF:EF + 128]),
        w1hj=cvt(w1cat[EF + 128:EF + 256]),
        wq1x=cvt(wq1x), wq2=cvt(inputs['hq_W2']),
        wk2=cvt(inputs['hk_W2']), wv2=cvt(inputs['hv_W2']),
        wn1ax=cvt(wn1a_x), wn1hx=cvt(wn1h_x), wn2=cvt(inputs['no_W2']),
    )
    hT16 = h.T.astype(np.float16)        # [128, N]

    bounds = np.searchsorted(dst_s, np.arange(NCORES + 1) * NPC)
    cnt = np.zeros((NCORES, W), np.int64)
    pc = []
    for c in range(NCORES):
        lo, hi = int(bounds[c]), int(bounds[c + 1])
        dl = dst_s[lo:hi] - c * NPC
        cnt[c] = np.bincount(dl >> 7, minlength=W)
        pc.append((lo, dl))
    T = np.maximum(1, (cnt.max(axis=0) + 127) // 128)
    T = ((T + 1) // 2) * 2              # even tile count per window (macro pairs)
    base_t = np.zeros(W + 1, np.int64)
    base_t[1:] = np.cumsum(T)
    Ttot = int(base_t[-1])
    EPAD = Ttot * 128

    in_maps = []
    for c in range(NCORES):
        lo, dl = pc[c]
        ncore = len(dl)
        # flat slot of each real edge within the padded tile layout
        ws = np.zeros(W + 1, np.int64)
        ws[1:] = np.cumsum(cnt[c])
        iw = np.arange(ncore)
        wofe = np.searchsorted(ws, iw, side='right') - 1   # window of each edge
        ein = iw - ws[wofe]                                # index within window
        slot = (base_t[wofe] + (ein >> 7)) * 128 + (ein & 127)

        hdstT = np.zeros((D, EPAD), np.float16)
        hdstT[:, slot] = hT16[:, dst_s[lo:lo + ncore]]
        hsrcT = np.zeros((D, EPAD), np.float16)
        hsrcT[:, slot] = hT16[:, src_s[lo:lo + ncore]]
        eft = np.zeros((EF, EPAD), np.float16)
        eft[:, slot] = ef_s[lo:lo + ncore].T.astype(np.float16)
        eww = np.zeros(EPAD, np.float32)
        eww[slot] = ew_s[lo:lo + ncore]

        nloc = dl - (wofe << 7)                            # node idx in window
        tile_of = slot >> 7
        p_of = slot & 127
        ohen = np.zeros((128, EPAD), np.float16)
        ohen[p_of, tile_of * 128 + nloc] = 1.0
        ohne = np.zeros((128, EPAD), np.float16)
        ohne[nloc, tile_of * 128 + p_of] = 1.0

        hoT = np.zeros((D, NPAD), np.float16)
        hoT[:, :NPC] = hT16[:, c * NPC:(c + 1) * NPC]
        m = dict(shared)
        m.update(
            hT_own=hoT,
            hdstT=hdstT, hsrcT=hsrcT, efT=eft,
            ohen=ohen, ohne=ohne,
            ew_c=np.ascontiguousarray(eww.reshape(Ttot, 128).T),
        )
        in_maps.append(m)
    return in_maps, [int(x) for x in T], [int(x) for x in base_t]


def _build(T, base_t):
    Ttot = base_t[-1]
    EPAD = Ttot * 128
    Tmax = max(T)
    nc = bass.Bass(target_bir_lowering=False, debug=False)
    dt = nc.dram_tensor
    hT_own_d = dt('hT_own', [128, NPAD], F16, kind='ExternalInput')
    hdstT_d = dt('hdstT', [128, EPAD], F16, kind='ExternalInput')
    hsrcT_d = dt('hsrcT', [128, EPAD], F16, kind='ExternalInput')
    efT_d = dt('efT', [EF, EPAD], F16, kind='ExternalInput')
    ohen_d = dt('ohen', [128, EPAD], F16, kind='ExternalInput')
    ohne_d = dt('ohne', [128, EPAD], F16, kind='ExternalInput')
    ew_d = dt('ew_c', [128, Ttot], F32, kind='ExternalInput')
    wd = {}
    for nm, p, q in [('w1ef', EF, 256), ('w1hi', 128, 256), ('w1hj', 128, 256),
                     ('wq1x', 128, 129), ('wq2', 128, 128), ('wk2', 128, 128),
                     ('wv2', 128, 128), ('wn1ax', 128, 129), ('wn1hx', 128, 129),
                     ('wn2', 128, 128)]:
        wd[nm] = dt(nm, [p, q], F16, kind='ExternalInput')
    out_d = dt('out', [NPAD, D], F32, kind='ExternalOutput')

    with ExitStack() as ctx:
        tc = ctx.enter_context(tile.TileContext(nc))
        cp = ctx.enter_context(tc.tile_pool(name='consts', bufs=1))

        ident = cp.tile([128, 128], F16, name='ident')
        make_identity(nc, ident[:])
        ebias_col = cp.tile([128, 1], F32, name='ebias_col')
        nc.gpsimd.memset(ebias_col[:], float(EXP_BIAS))
        ln025_col = cp.tile([128, 1], F32, name='ln025_col')
        nc.gpsimd.memset(ln025_col[:], float(np.log(0.25)))

        wsb = {}
        for nm, dr in wd.items():
            t = cp.tile(list(dr.shape), F16, name=nm + '_s')
            nc.sync.dma_start(out=t[:], in_=dr[:])
            wsb[nm] = t

        ew_s = cp.tile([128, Ttot], F32, name='ew_s')
        nc.sync.dma_start(out=ew_s[:], in_=ew_d[:])
        hT_own = cp.tile([128, NPAD], F16, name='hT_own')
        nc.sync.dma_start(out=hT_own[:], in_=hT_own_d[:])

        wp = ctx.enter_context(tc.tile_pool(name='wp', bufs=4))
        tp = ctx.enter_context(tc.tile_pool(name='tp', bufs=6))
        pp = ctx.enter_context(tc.tile_pool(name='pp', bufs=2, space='PSUM'))
        ph = ctx.enter_context(tc.tile_pool(name='ph', bufs=2, space='PSUM'))
        ac = ctx.enter_context(tc.tile_pool(name='ac', bufs=2, space='PSUM'))

        def load_window(w):
            Tw, tb = T[w], base_t[w]
            s = {'w': w, 'Tw': Tw, 'tb': tb, 'NM': Tw // 2,
                 'hTw': hT_own[:, w * 128:(w + 1) * 128]}
            LW = Tw * 128
            for nm, dr, P in [('hdstT_w', hdstT_d, 128), ('hsrcT_w', hsrcT_d, 128),
                              ('eft_w', efT_d, EF), ('ohen_w', ohen_d, 128),
                              ('ohne_w', ohne_d, 128)]:
                t = wp.tile([P, Tmax * 128], F16, name=nm)
                nc.sync.dma_start(out=t[:, 0:LW], in_=dr[:, tb * 128:tb * 128 + LW])
                s[nm] = t
            # ---- q-MLP for this window (LN rstd folded out) ----
            qps_t = ph.tile([128, 512], F32, name='hdn')
            qps = qps_t[:, 0:129]
            nc.tensor.matmul(qps, s['hTw'], wsb['wq1x'][:], start=True, stop=True)
            qsb = wp.tile([128, 129], F32, name='qsb')
            nc.vector.tensor_copy(qsb[:], qps)
            qst = wp.tile([128, 7], F32, name='qst')
            # 0 ssq, 1 negmu, 2 mu2, 3 s128e, 4 var_e, 5 lnv, 6 rstd
            qscr = wp.tile([128, 128], F16, name='qscr')
            nc.scalar.activation(qscr[:], qsb[:, 0:128], AF.Square,
                                 accum_out=qst[:, 0:1])
            nc.gpsimd.tensor_scalar(qst[:, 1:2], qsb[:, 128:129], -1.0 / 128, None,
                                    op0=ALU.mult)
            nc.gpsimd.tensor_tensor(qst[:, 2:3], qst[:, 1:2], qst[:, 1:2],
                                    op=ALU.mult)
            nc.gpsimd.tensor_scalar(qst[:, 3:4], qst[:, 0:1], 1.0 / 128, float(EPS),
                                    op0=ALU.mult, op1=ALU.add)
            nc.gpsimd.tensor_tensor(qst[:, 4:5], qst[:, 3:4], qst[:, 2:3],
                                    op=ALU.subtract)
            nc.scalar.activation(qst[:, 5:6], qst[:, 4:5], AF.Ln)
            nc.scalar.activation(qst[:, 6:7], qst[:, 5:6], AF.Exp, scale=-0.5)
            qrelu = wp.tile([128, 128], F16, name='qrelu')
            nc.scalar.activation(qrelu[:], qsb[:, 0:128], AF.Relu, bias=qst[:, 1:2])
            kvps_t = pp.tile([128, 512], F16, name='kvT')
            qrT_ps = kvps_t[:, 0:128]
            nc.tensor.transpose(qrT_ps, qrelu[:], ident[:])
            qrT = wp.tile([128, 128], F16, name='qrT')
            nc.vector.tensor_copy(qrT[:], qrT_ps)
            qf_t = pp.tile([128, 512], F32, name='kv')
            qf_ps = qf_t[:, 0:128]
            nc.tensor.matmul(qf_ps, qrT[:], wsb['wq2'][:], start=True, stop=True)
            q_win = wp.tile([128, 128], F16, name='q_win')
            nc.scalar.activation(q_win[:], qf_ps, AF.Copy, scale=qst[:, 6:7])
            s['q_win'] = q_win
            s['acc'] = ac.tile([128, 136], F32, name='acc')
            return s

        def emit_macro(s, m):
            t0 = 2 * m
            gt = s['tb'] + t0
            sl = lambda g: slice((t0 + g) * 128, (t0 + g + 1) * 128)
            hdn = ph.tile([128, 512], F32, name='hdn')
            for g in range(2):
                hv = hdn[:, g * 256:(g + 1) * 256]
                nc.tensor.matmul(hv, s['eft_w'][:, sl(g)], wsb['w1ef'][:],
                                 start=True, stop=False)
                nc.tensor.matmul(hv, s['hdstT_w'][:, sl(g)], wsb['w1hi'][:],
                                 start=False, stop=False)
                nc.tensor.matmul(hv, s['hsrcT_w'][:, sl(g)], wsb['w1hj'][:],
                                 start=False, stop=True)
            # stats (storage k0 v0 k1 v1): 0:4 sums, 4:8 negmu, 8:12 mu2,
            # 12:16 ssq, 16:20 s128e, 20:24 var_e, 24:28 lnv, 28:32 rstd
            # reduces/Square/relu read hdn PSUM directly (no x_sb copy);
            # wide elementwise stays OFF gpsimd (~2us/op there vs ~0.2us DVE)
            st = tp.tile([128, 32], F32, name='st')
            x4 = hdn[:].rearrange('p (q c) -> p q c', c=128)
            nc.vector.reduce_sum(out=st[:, 0:4], in_=x4, axis=AX.X)
            sq = tp.tile([128, 512], F16, name='sq')
            nc.scalar.activation(sq[:], hdn[:], AF.Square)
            nc.vector.reduce_sum(
                out=st[:, 12:16],
                in_=sq[:].rearrange('p (q c) -> p q c', c=128), axis=AX.X)
            nc.gpsimd.tensor_scalar(st[:, 4:8], st[:, 0:4], -1.0 / 128, None,
                                    op0=ALU.mult)
            nc.gpsimd.tensor_tensor(st[:, 8:12], st[:, 4:8], st[:, 4:8],
                                    op=ALU.mult)
            nc.gpsimd.tensor_scalar(st[:, 16:20], st[:, 12:16], 1.0 / 128,
                                    float(EPS), op0=ALU.mult, op1=ALU.add)
            nc.gpsimd.tensor_tensor(st[:, 20:24], st[:, 16:20], st[:, 8:12],
                                    op=ALU.subtract)
            nc.scalar.activation(st[:, 24:28], st[:, 20:24], AF.Ln)
            nc.scalar.activation(st[:, 28:32], st[:, 24:28], AF.Exp, scale=-0.5)
            # relu(x - mu) in f16 (rstd folded out)
            relu1 = tp.tile([128, 512], F16, name='relu1')
            for g in range(4):
                if g % 2 == 0:
                    nc.vector.tensor_scalar(
                        relu1[:, g * 128:(g + 1) * 128],
                        hdn[:, g * 128:(g + 1) * 128],
                        st[:, 4 + g:5 + g], 0.0, op0=ALU.add, op1=ALU.max)
                else:
                    nc.scalar.activation(
                        relu1[:, g * 128:(g + 1) * 128],
                        hdn[:, g * 128:(g + 1) * 128],
                        AF.Relu, bias=st[:, 4 + g:5 + g])
            kvq_t = pp.tile([128, 1024], F16, name='kvT')
            kvT_ps = kvq_t[:, 0:512]
            for g in range(4):
                nc.tensor.transpose(kvT_ps[:, g * 128:(g + 1) * 128],
                                    relu1[:, g * 128:(g + 1) * 128], ident[:])
            kvT = tp.tile([128, 512], F16, name='kvT_sb')
            nc.scalar.activation(kvT[:], kvT_ps[:], AF.Copy)
            kv = pp.tile([128, 512], F32, name='kv')
            for g, wnm in enumerate(['wk2', 'wv2', 'wk2', 'wv2']):
                nc.tensor.matmul(kv[:, g * 128:(g + 1) * 128],
                                 kvT[:, g * 128:(g + 1) * 128], wsb[wnm][:],
                                 start=True, stop=True)
            kv3 = kv[:].rearrange('p (t c) -> p t c', c=256)
            # qe = q[dst] via one-hot matmuls (shares the kvT bank; frees hdn early)
            qe_ps = kvq_t[:, 512:1024].bitcast(F32)
            for g in range(2):
                nc.tensor.matmul(qe_ps[:, g * 128:(g + 1) * 128],
                                 s['ohne_w'][:, sl(g)], s['q_win'][:],
                                 start=True, stop=True)
            qe_sb = tp.tile([128, 256], F16, name='qe_sb')
            nc.scalar.activation(qe_sb[:], qe_ps, AF.Copy)
            # logits
            qk = tp.tile([128, 256], F16, name='qk')
            nc.vector.tensor_tensor(
                qk[:].rearrange('p (t c) -> p t c', c=128),
                qe_sb[:].rearrange('p (t c) -> p t c', c=128),
                kv3[:, :, 0:128], op=ALU.mult)
            lg = tp.tile([128, 16], F32, name='lg')
            nc.vector.reduce_sum(
                out=lg[:], in_=qk[:].rearrange('p (th d) -> p th d', d=16),
                axis=AX.X)
            rstd2 = st[:, 28:32].rearrange('p (t g) -> p t g', g=2)
            lgs = tp.tile([128, 16], F32, name='lgs')
            nc.gpsimd.tensor_tensor(
                lgs[:].rearrange('p (t h) -> p t h', h=NH),
                lg[:].rearrange('p (t h) -> p t h', h=NH),
                rstd2[:, :, 0:1].to_broadcast([128, 2, NH]), op=ALU.mult)
            exw = tp.tile([128, 16], F32, name='exw')
            nc.scalar.activation(exw[:], lgs[:], AF.Exp, scale=0.25,
                                 bias=ebias_col[:])
            wr = tp.tile([128, 18], F32, name='wr')
            nc.gpsimd.tensor_tensor(
                wr[:, 16:18].rearrange('p (t o) -> p t o', o=1),
                ew_s[:, gt:gt + 2].rearrange('p (t o) -> p t o', o=1),
                rstd2[:, :, 1:2], op=ALU.mult)
            nc.gpsimd.tensor_tensor(
                wr[:, 0:16].rearrange('p (t h) -> p t h', h=NH),
                exw[:].rearrange('p (t h) -> p t h', h=NH),
                wr[:, 16:18].unsqueeze(2).to_broadcast([128, 2, NH]),
                op=ALU.mult)
            # X = [v_raw * (ex*ew*rstd_v) | ex]
            X = tp.tile([128, 272], F16, name='X')
            X3 = X[:].rearrange('p (t c) -> p t c', c=136)
            nc.vector.tensor_tensor(
                X3[:, :, 0:128].rearrange('p t (h d) -> p t h d', d=DH),
                kv3[:, :, 128:256].rearrange('p t (h d) -> p t h d', d=DH),
                wr[:, 0:16].rearrange('p (t h) -> p t h', h=NH).unsqueeze(
                    3).to_broadcast([128, 2, NH, DH]), op=ALU.mult)
            nc.gpsimd.tensor_copy(
                X3[:, :, 128:136],
                exw[:].rearrange('p (t h) -> p t h', h=NH))
            for i in range(2):
                nc.tensor.matmul(s['acc'][:], s['ohen_w'][:, sl(i)],
                                 X[:, i * 136:(i + 1) * 136],
                                 start=(m == 0 and i == 0),
                                 stop=(m == s['NM'] - 1 and i == 1))

        def emit_tail(s):
            w = s['w']
            acc = s['acc']
            den = wp.tile([128, 8], F32, name='den')
            nc.vector.tensor_scalar(den[:], acc[:, 128:136], 1e-30, None,
                                    op0=ALU.max)
            rden = wp.tile([128, 8], F32, name='rden')
            nc.vector.reciprocal(rden[:], den[:])
            attn = wp.tile([128, 128], F16, name='attn')
            nc.vector.tensor_tensor(
                attn[:].rearrange('p (h d) -> p h d', d=DH),
                acc[:, 0:128].rearrange('p (h d) -> p h d', d=DH),
                rden[:].unsqueeze(2).to_broadcast([128, NH, DH]), op=ALU.mult)
            Ah = pp.tile([128, 1024], F16, name='kvT')
            attnT_ps = Ah[:, 0:128]
            nc.tensor.transpose(attnT_ps, attn[:], ident[:])
            attnT = wp.tile([128, 128], F16, name='attnT')
            nc.vector.tensor_copy(attnT[:], attnT_ps)
            An = ph.tile([128, 512], F32, name='hdn')
            no_ps = An[:, 0:129]
            nc.tensor.matmul(no_ps, attnT[:], wsb['wn1ax'][:], start=True,
                             stop=False)
            nc.tensor.matmul(no_ps, s['hTw'], wsb['wn1hx'][:], start=False,
                             stop=True)
            nosb = wp.tile([128, 129], F32, name='nosb')
            nc.vector.tensor_copy(nosb[:], no_ps)
            scr2 = wp.tile([128, 128], F16, name='scr2')
            stn = wp.tile([128, 7], F32, name='stn')
            nc.scalar.activation(scr2[:], nosb[:, 0:128], AF.Square,
                                 accum_out=stn[:, 0:1])
            nc.gpsimd.tensor_scalar(stn[:, 1:2], nosb[:, 128:129], -1.0 / 128,
                                    None, op0=ALU.mult)
            nc.gpsimd.tensor_tensor(stn[:, 2:3], stn[:, 1:2], stn[:, 1:2],
                                    op=ALU.mult)
            nc.gpsimd.tensor_scalar(stn[:, 3:4], stn[:, 0:1], 1.0 / 128,
                                    float(EPS), op0=ALU.mult, op1=ALU.add)
            nc.gpsimd.tensor_tensor(stn[:, 4:5], stn[:, 3:4], stn[:, 2:3],
                                    op=ALU.subtract)
            nc.scalar.activation(stn[:, 5:6], stn[:, 4:5], AF.Ln)
            nc.scalar.activation(stn[:, 6:7], stn[:, 5:6], AF.Exp, scale=-0.5)
            norelu = wp.tile([128, 128], F16, name='norelu')
            nc.scalar.activation(norelu[:], no_ps[:, 0:128], AF.Relu,
                                 bias=stn[:, 1:2])
            norT_ps = Ah[:, 128:256]
            nc.tensor.transpose(norT_ps, norelu[:], ident[:])
            norT = wp.tile([128, 128], F16, name='norT')
            nc.vector.tensor_copy(norT[:], norT_ps)
            out_ps = pp.tile([128, 512], F32, name='kv')
            nc.tensor.matmul(out_ps[:, 0:128], norT[:], wsb['wn2'][:], start=True,
                             stop=True)
            out_sb = wp.tile([128, 128], F32, name='out_sb')
            nc.scalar.activation(out_sb[:], out_ps[:, 0:128], AF.Copy,
                                 scale=stn[:, 6:7])
            nc.sync.dma_start(out=out_d[w * 128:(w + 1) * 128, :], in_=out_sb[:])

        worder = sorted(range(W), key=lambda w: (T[w], w))
        loaded = [load_window(worder[j]) for j in (0, 1) if j < W]
        for i0 in range(0, W, 2):
            ws = loaded
            loaded = [load_window(worder[j]) for j in (i0 + 2, i0 + 3) if j < W]
            for m in range(max(s['NM'] for s in ws)):
                for s in ws:
                    if m < s['NM']:
                        emit_macro(s, m)
            for s in ws:
                emit_tail(s)
    return nc


def kernel(_trace=False, **inputs):
    import bass_rust
    in_maps, T, base_t = _preprocess(inputs)
    nc = _build(T, base_t)
    bass_rust.generate_event_semaphores(nc)
    res = bass_utils.run_bass_kernel_spmd(nc, in_maps, core_ids=list(range(NCORES)),
                                          trace=_trace)
    out = np.concatenate(
        [np.asarray(res.results[c]['out'])[:NPC] for c in range(NCORES)], axis=0)
    if _trace:
        return out.astype(np.float32), res
    return out.astype(np.float32)



# revision 30
# speedup vs baseline: 35.6357x; 1.0092x over previous
import numpy as np
from contextlib import ExitStack

import concourse.bass as bass
import concourse.tile as tile
from concourse import mybir, bass_utils
from concourse.masks import make_identity

N, E, D, EF = 50000, 800000, 128, 64
NH, DH = 8, 16
NCORES = 8
NPC = N // NCORES           # 6250 nodes per core
W = 49                      # windows of 128 nodes per core
NPAD = W * 128              # 6272
EPS = 1e-5
EXP_BIAS = -2.7726          # exp scaled by 2^-4; cancels in ws/den ratio

F32 = mybir.dt.float32
F16 = mybir.dt.float16
I32 = mybir.dt.int32
AF = mybir.ActivationFunctionType
ALU = mybir.AluOpType
AX = mybir.AxisListType


def _preprocess(inputs):
    f32 = lambda x: np.ascontiguousarray(np.asarray(x, np.float32))
    h = f32(inputs['h'])
    ef = f32(inputs['edge_feat'])
    e_w = f32(inputs['e_w'])
    src = np.asarray(inputs['edge_index'][0], np.int64)
    dst = np.asarray(inputs['edge_index'][1], np.int64)

    order = np.argsort(dst, kind='stable')
    src_s, dst_s = src[order], dst[order]
    ew_s, ef_s = e_w[order], ef[order]

    w1cat = np.concatenate([f32(inputs['hk_W1']), f32(inputs['hv_W1'])], axis=1)
    wq1 = f32(inputs['hq_W1'])
    wq1x = np.concatenate([wq1, wq1.sum(1, keepdims=True)], 1)   # [128, 129]
    wn1 = f32(inputs['no_W1'])
    wn1a_x = np.concatenate([wn1[0:128], wn1[0:128].sum(1, keepdims=True)], 1)
    wn1h_x = np.concatenate([wn1[128:256], wn1[128:256].sum(1, keepdims=True)], 1)

    cvt = lambda x: np.ascontiguousarray(np.asarray(x, np.float16))
    shared = dict(
        w1ef=cvt(w1cat[0:EF]), w1hi=cvt(w1cat[EF:EF + 128]),
        w1hj=cvt(w1cat[EF + 128:EF + 256]),
        wq1x=cvt(wq1x), wq2=cvt(inputs['hq_W2']),
        wk2=cvt(inputs['hk_W2']), wv2=cvt(inputs['hv_W2']),
        wn1ax=cvt(wn1a_x), wn1hx=cvt(wn1h_x), wn2=cvt(inputs['no_W2']),
    )
    hT16 = h.T.astype(np.float16)        # [128, N]

    bounds = np.searchsorted(dst_s, np.arange(NCORES + 1) * NPC)
    cnt = np.zeros((NCORES, W), np.int64)
    pc = []
    for c in range(NCORES):
        lo, hi = int(bounds[c]), int(bounds[c + 1])
        dl = dst_s[lo:hi] - c * NPC
        cnt[c] = np.bincount(dl >> 7, minlength=W)
        pc.append((lo, dl))
    T = np.maximum(1, (cnt.max(axis=0) + 127) // 128)
    T = ((T + 1) // 2) * 2              # even tile count per window (macro pairs)
    base_t = np.zeros(W + 1, np.int64)
    base_t[1:] = np.cumsum(T)
    Ttot = int(base_t[-1])
    EPAD = Ttot * 128

    in_maps = []
    for c in range(NCORES):
        lo, dl = pc[c]
        ncore = len(dl)
        # flat slot of each real edge within the padded tile layout
        ws = np.zeros(W + 1, np.int64)
        ws[1:] = np.cumsum(cnt[c])
        iw = np.arange(ncore)
        wofe = np.searchsorted(ws, iw, side='right') - 1   # window of each edge
        ein = iw - ws[wofe]                                # index within window
        slot = (base_t[wofe] + (ein >> 7)) * 128 + (ein & 127)

        hdstT = np.zeros((D, EPAD), np.float16)
        hdstT[:, slot] = hT16[:, dst_s[lo:lo + ncore]]
        hsrcT = np.zeros((D, EPAD), np.float16)
        hsrcT[:, slot] = hT16[:, src_s[lo:lo + ncore]]
        eft = np.zeros((EF, EPAD), np.float16)
        eft[:, slot] = ef_s[lo:lo + ncore].T.astype(np.float16)
        eww = np.zeros(EPAD, np.float32)
        eww[slot] = ew_s[lo:lo + ncore]

        nloc = dl - (wofe << 7)                            # node idx in window
        tile_of = slot >> 7
        p_of = slot & 127
        ohen = np.zeros((128, EPAD), np.float16)
        ohen[p_of, tile_of * 128 + nloc] = 1.0
        ohne = np.zeros((128, EPAD), np.float16)
        ohne[nloc, tile_of * 128 + p_of] = 1.0

        hoT = np.zeros((D, NPAD), np.float16)
        hoT[:, :NPC] = hT16[:, c * NPC:(c + 1) * NPC]
        m = dict(shared)
        m.update(
            hT_own=hoT,
            hdstT=hdstT, hsrcT=hsrcT, efT=eft,
            ohen=ohen, ohne=ohne,
            ew_c=np.ascontiguousarray(eww.reshape(Ttot, 128).T),
        )
        in_maps.append(m)
    return in_maps, [int(x) for x in T], [int(x) for x in base_t]


def _build(T, base_t):
    Ttot = base_t[-1]
    EPAD = Ttot * 128
    Tmax = max(T)
    nc = bass.Bass(target_bir_lowering=False, debug=False)
    dt = nc.dram_tensor
    hT_own_d = dt('hT_own', [128, NPAD], F16, kind='ExternalInput')
    hdstT_d = dt('hdstT', [128, EPAD], F16, kind='ExternalInput')
    hsrcT_d = dt('hsrcT', [128, EPAD], F16, kind='ExternalInput')
    efT_d = dt('efT', [EF, EPAD], F16, kind='ExternalInput')
    ohen_d = dt('ohen', [128, EPAD], F16, kind='ExternalInput')
    ohne_d = dt('ohne', [128, EPAD], F16, kind='ExternalInput')
    ew_d = dt('ew_c', [128, Ttot], F32, kind='ExternalInput')
    wd = {}
    for nm, p, q in [('w1ef', EF, 256), ('w1hi', 128, 256), ('w1hj', 128, 256),
                     ('wq1x', 128, 129), ('wq2', 128, 128), ('wk2', 128, 128),
                     ('wv2', 128, 128), ('wn1ax', 128, 129), ('wn1hx', 128, 129),
                     ('wn2', 128, 128)]:
        wd[nm] = dt(nm, [p, q], F16, kind='ExternalInput')
    out_d = dt('out', [NPAD, D], F32, kind='ExternalOutput')

    with ExitStack() as ctx:
        tc = ctx.enter_context(tile.TileContext(nc))
        cp = ctx.enter_context(tc.tile_pool(name='consts', bufs=1))

        ident = cp.tile([128, 128], F16, name='ident')
        make_identity(nc, ident[:])
        ebias_col = cp.tile([128, 1], F32, name='ebias_col')
        nc.gpsimd.memset(ebias_col[:], float(EXP_BIAS))
        ln025_col = cp.tile([128, 1], F32, name='ln025_col')
        nc.gpsimd.memset(ln025_col[:], float(np.log(0.25)))

        wsb = {}
        for nm, dr in wd.items():
            t = cp.tile(list(dr.shape), F16, name=nm + '_s')
            nc.sync.dma_start(out=t[:], in_=dr[:])
            wsb[nm] = t

        ew_s = cp.tile([128, Ttot], F32, name='ew_s')
        nc.sync.dma_start(out=ew_s[:], in_=ew_d[:])
        hT_own = cp.tile([128, NPAD], F16, name='hT_own')
        nc.sync.dma_start(out=hT_own[:], in_=hT_own_d[:])

        wp = ctx.enter_context(tc.tile_pool(name='wp', bufs=4))
        tp = ctx.enter_context(tc.tile_pool(name='tp', bufs=6))
        pp = ctx.enter_context(tc.tile_pool(name='pp', bufs=2, space='PSUM'))
        ph = ctx.enter_context(tc.tile_pool(name='ph', bufs=2, space='PSUM'))
        ac = ctx.enter_context(tc.tile_pool(name='ac', bufs=2, space='PSUM'))

        def load_window(w):
            Tw, tb = T[w], base_t[w]
            s = {'w': w, 'Tw': Tw, 'tb': tb, 'NM': Tw // 2,
                 'hTw': hT_own[:, w * 128:(w + 1) * 128]}
            LW = Tw * 128
            for nm, dr, P in [('hdstT_w', hdstT_d, 128), ('hsrcT_w', hsrcT_d, 128),
                              ('eft_w', efT_d, EF), ('ohen_w', ohen_d, 128),
                              ('ohne_w', ohne_d, 128)]:
                t = wp.tile([P, Tmax * 128], F16, name=nm)
                nc.sync.dma_start(out=t[:, 0:LW], in_=dr[:, tb * 128:tb * 128 + LW])
                s[nm] = t
            # ---- q-MLP for this window (LN rstd folded out) ----
            qps_t = ph.tile([128, 512], F32, name='hdn')
            qps = qps_t[:, 0:129]
            nc.tensor.matmul(qps, s['hTw'], wsb['wq1x'][:], start=True, stop=True)
            qsb = wp.tile([128, 129], F32, name='qsb')
            nc.vector.tensor_copy(qsb[:], qps)
            qst = wp.tile([128, 7], F32, name='qst')
            # 0 ssq, 1 negmu, 2 mu2, 3 s128e, 4 var_e, 5 lnv, 6 rstd
            qscr = wp.tile([128, 128], F16, name='qscr')
            nc.scalar.activation(qscr[:], qsb[:, 0:128], AF.Square,
                                 accum_out=qst[:, 0:1])
            nc.gpsimd.tensor_scalar(qst[:, 1:2], qsb[:, 128:129], -1.0 / 128, None,
                                    op0=ALU.mult)
            nc.gpsimd.tensor_tensor(qst[:, 2:3], qst[:, 1:2], qst[:, 1:2],
                                    op=ALU.mult)
            nc.gpsimd.tensor_scalar(qst[:, 3:4], qst[:, 0:1], 1.0 / 128, float(EPS),
                                    op0=ALU.mult, op1=ALU.add)
            nc.gpsimd.tensor_tensor(qst[:, 4:5], qst[:, 3:4], qst[:, 2:3],
                                    op=ALU.subtract)
            nc.scalar.activation(qst[:, 5:6], qst[:, 4:5], AF.Ln)
            nc.scalar.activation(qst[:, 6:7], qst[:, 5:6], AF.Exp, scale=-0.5)
            qrelu = wp.tile([128, 128], F16, name='qrelu')
            nc.scalar.activation(qrelu[:], qsb[:, 0:128], AF.Relu, bias=qst[:, 1:2])
            kvps_t = pp.tile([128, 512], F16, name='kvT')
            qrT_ps = kvps_t[:, 0:128]
            nc.tensor.transpose(qrT_ps, qrelu[:], ident[:])
            qrT = wp.tile([128, 128], F16, name='qrT')
            nc.vector.tensor_copy(qrT[:], qrT_ps)
            qf_t = pp.tile([128, 512], F32, name='kv')
            qf_ps = qf_t[:, 0:128]
            nc.tensor.matmul(qf_ps, qrT[:], wsb['wq2'][:], start=True, stop=True)
            q_win = wp.tile([128, 128], F16, name='q_win')
            nc.scalar.activation(q_win[:], qf_ps, AF.Copy, scale=qst[:, 6:7])
            s['q_win'] = q_win
            s['acc'] = ac.tile([128, 136], F32, name='acc')
            return s

        def emit_macro(s, m):
            t0 = 2 * m
            gt = s['tb'] + t0
            sl = lambda g: slice((t0 + g) * 128, (t0 + g + 1) * 128)
            # f16 PSUM: same bank footprint as [128,512] f32, but 2x-rate
            # DVE reads for the reduces/relus below
            hdn_t = ph.tile([128, 1024], F16, name='hdn')
            hdn = hdn_t[:, 0:512]
            for g in range(2):
                hv = hdn[:, g * 256:(g + 1) * 256]
                nc.tensor.matmul(hv, s['eft_w'][:, sl(g)], wsb['w1ef'][:],
                                 start=True, stop=False)
                nc.tensor.matmul(hv, s['hdstT_w'][:, sl(g)], wsb['w1hi'][:],
                                 start=False, stop=False)
                nc.tensor.matmul(hv, s['hsrcT_w'][:, sl(g)], wsb['w1hj'][:],
                                 start=False, stop=True)
            # stats (storage k0 v0 k1 v1): 0:4 sums, 4:8 negmu, 8:12 mu2,
            # 12:16 ssq, 16:20 s128e, 20:24 var_e, 24:28 lnv, 28:32 rstd
            # reduces/Square/relu read hdn PSUM directly (no x_sb copy);
            # wide elementwise stays OFF gpsimd (~2us/op there vs ~0.2us DVE)
            st = tp.tile([128, 32], F32, name='st')
            x4 = hdn[:].rearrange('p (q c) -> p q c', c=128)
            nc.vector.reduce_sum(out=st[:, 0:4], in_=x4, axis=AX.X)
            sq = tp.tile([128, 512], F16, name='sq')
            nc.scalar.activation(sq[:], hdn[:], AF.Square)
            nc.vector.reduce_sum(
                out=st[:, 12:16],
                in_=sq[:].rearrange('p (q c) -> p q c', c=128), axis=AX.X)
            nc.gpsimd.tensor_scalar(st[:, 4:8], st[:, 0:4], -1.0 / 128, None,
                                    op0=ALU.mult)
            nc.gpsimd.tensor_tensor(st[:, 8:12], st[:, 4:8], st[:, 4:8],
                                    op=ALU.mult)
            nc.gpsimd.tensor_scalar(st[:, 16:20], st[:, 12:16], 1.0 / 128,
                                    float(EPS), op0=ALU.mult, op1=ALU.add)
            nc.gpsimd.tensor_tensor(st[:, 20:24], st[:, 16:20], st[:, 8:12],
                                    op=ALU.subtract)
            nc.scalar.activation(st[:, 24:28], st[:, 20:24], AF.Ln)
            nc.scalar.activation(st[:, 28:32], st[:, 24:28], AF.Exp, scale=-0.5)
            # relu(x - mu) in f16 (rstd folded out)
            relu1 = tp.tile([128, 512], F16, name='relu1')
            for g in range(4):
                if g % 2 == 0:
                    nc.vector.tensor_scalar(
                        relu1[:, g * 128:(g + 1) * 128],
                        hdn[:, g * 128:(g + 1) * 128],
                        st[:, 4 + g:5 + g], 0.0, op0=ALU.add, op1=ALU.max)
                else:
                    nc.scalar.activation(
                        relu1[:, g * 128:(g + 1) * 128],
                        hdn[:, g * 128:(g + 1) * 128],
                        AF.Relu, bias=st[:, 4 + g:5 + g])
            kvq_t = pp.tile([128, 1024], F16, name='kvT')
            kvT_ps = kvq_t[:, 0:512]
            for g in range(4):
                nc.tensor.transpose(kvT_ps[:, g * 128:(g + 1) * 128],
                                    relu1[:, g * 128:(g + 1) * 128], ident[:])
            kvT = tp.tile([128, 512], F16, name='kvT_sb')
            nc.scalar.activation(kvT[:], kvT_ps[:], AF.Copy)
            kv_t = pp.tile([128, 1024], F16, name='kv')
            kv = kv_t[:, 0:512]
            for g, wnm in enumerate(['wk2', 'wv2', 'wk2', 'wv2']):
                nc.tensor.matmul(kv[:, g * 128:(g + 1) * 128],
                                 kvT[:, g * 128:(g + 1) * 128], wsb[wnm][:],
                                 start=True, stop=True)
            kv3 = kv.rearrange('p (t c) -> p t c', c=256)
            # qe = q[dst] via one-hot matmuls (shares the kvT bank; frees hdn early)
            qe_ps = kvq_t[:, 512:1024].bitcast(F32)
            for g in range(2):
                nc.tensor.matmul(qe_ps[:, g * 128:(g + 1) * 128],
                                 s['ohne_w'][:, sl(g)], s['q_win'][:],
                                 start=True, stop=True)
            qe_sb = tp.tile([128, 256], F16, name='qe_sb')
            nc.scalar.activation(qe_sb[:], qe_ps, AF.Copy)
            # logits
            qk = tp.tile([128, 256], F16, name='qk')
            nc.vector.tensor_tensor(
                qk[:].rearrange('p (t c) -> p t c', c=128),
                qe_sb[:].rearrange('p (t c) -> p t c', c=128),
                kv3[:, :, 0:128], op=ALU.mult)
            lg = tp.tile([128, 16], F32, name='lg')
            nc.vector.reduce_sum(
                out=lg[:], in_=qk[:].rearrange('p (th d) -> p th d', d=16),
                axis=AX.X)
            rstd2 = st[:, 28:32].rearrange('p (t g) -> p t g', g=2)
            lgs = tp.tile([128, 16], F32, name='lgs')
            nc.gpsimd.tensor_tensor(
                lgs[:].rearrange('p (t h) -> p t h', h=NH),
                lg[:].rearrange('p (t h) -> p t h', h=NH),
                rstd2[:, :, 0:1].to_broadcast([128, 2, NH]), op=ALU.mult)
            # X = [v_raw * (ex*ew*rstd_v) | ex]; exw Exp writes its f16 copy
            # straight into X's denominator columns (no separate copy op)
            X = tp.tile([128, 272], F16, name='X')
            X3 = X[:].rearrange('p (t c) -> p t c', c=136)
            nc.scalar.activation(X3[:, :, 128:136], lgs[:].rearrange(
                'p (t h) -> p t h', h=NH), AF.Exp, scale=0.25,
                bias=ebias_col[:])
            exw = X3[:, :, 128:136]
            wr = tp.tile([128, 18], F16, name='wr')
            nc.gpsimd.tensor_tensor(
                wr[:, 16:18].rearrange('p (t o) -> p t o', o=1),
                ew_s[:, gt:gt + 2].rearrange('p (t o) -> p t o', o=1),
                rstd2[:, :, 1:2], op=ALU.mult)
            nc.gpsimd.tensor_tensor(
                wr[:, 0:16].rearrange('p (t h) -> p t h', h=NH),
                exw,
                wr[:, 16:18].unsqueeze(2).to_broadcast([128, 2, NH]),
                op=ALU.mult)
            nc.vector.tensor_tensor(
                X3[:, :, 0:128].rearrange('p t (h d) -> p t h d', d=DH),
                kv3[:, :, 128:256].rearrange('p t (h d) -> p t h d', d=DH),
                wr[:, 0:16].rearrange('p (t h) -> p t h', h=NH).unsqueeze(
                    3).to_broadcast([128, 2, NH, DH]), op=ALU.mult)
            for i in range(2):
                nc.tensor.matmul(s['acc'][:], s['ohen_w'][:, sl(i)],
                                 X[:, i * 136:(i + 1) * 136],
                                 start=(m == 0 and i == 0),
                                 stop=(m == s['NM'] - 1 and i == 1))

        def emit_tail(s):
            w = s['w']
            acc = s['acc']
            den = wp.tile([128, 8], F32, name='den')
            nc.vector.tensor_scalar(den[:], acc[:, 128:136], 1e-30, None,
                                    op0=ALU.max)
            rden = wp.tile([128, 8], F32, name='rden')
            nc.vector.reciprocal(rden[:], den[:])
            attn = wp.tile([128, 128], F16, name='attn')
            nc.vector.tensor_tensor(
                attn[:].rearrange('p (h d) -> p h d', d=DH),
                acc[:, 0:128].rearrange('p (h d) -> p h d', d=DH),
                rden[:].unsqueeze(2).to_broadcast([128, NH, DH]), op=ALU.mult)
            Ah = pp.tile([128, 1024], F16, name='kvT')
            attnT_ps = Ah[:, 0:128]
            nc.tensor.transpose(attnT_ps, attn[:], ident[:])
            attnT = wp.tile([128, 128], F16, name='attnT')
            nc.vector.tensor_copy(attnT[:], attnT_ps)
            An = ph.tile([128, 512], F32, name='hdn')
            no_ps = An[:, 0:129]
            nc.tensor.matmul(no_ps, attnT[:], wsb['wn1ax'][:], start=True,
                             stop=False)
            nc.tensor.matmul(no_ps, s['hTw'], wsb['wn1hx'][:], start=False,
                             stop=True)
            nosb = wp.tile([128, 129], F32, name='nosb')
            nc.vector.tensor_copy(nosb[:], no_ps)
            scr2 = wp.tile([128, 128], F16, name='scr2')
            stn = wp.tile([128, 7], F32, name='stn')
            nc.scalar.activation(scr2[:], nosb[:, 0:128], AF.Square,
                                 accum_out=stn[:, 0:1])
            nc.gpsimd.tensor_scalar(stn[:, 1:2], nosb[:, 128:129], -1.0 / 128,
                                    None, op0=ALU.mult)
            nc.gpsimd.tensor_tensor(stn[:, 2:3], stn[:, 1:2], stn[:, 1:2],
                                    op=ALU.mult)
            nc.gpsimd.tensor_scalar(stn[:, 3:4], stn[:, 0:1], 1.0 / 128,
                                    float(EPS), op0=ALU.mult, op1=ALU.add)
            nc.gpsimd.tensor_tensor(stn[:, 4:5], stn[:, 3:4], stn[:, 2:3],
                                    op=ALU.subtract)
            nc.scalar.activation(stn[:, 5:6], stn[:, 4:5], AF.Ln)
            nc.scalar.activation(stn[:, 6:7], stn[:, 5:6], AF.Exp, scale=-0.5)
            norelu = wp.tile([128, 128], F16, name='norelu')
            nc.scalar.activation(norelu[:], no_ps[:, 0:128], AF.Relu,
                                 bias=stn[:, 1:2])
            norT_ps = Ah[:, 128:256]
            nc.tensor.transpose(norT_ps, norelu[:], ident[:])
            norT = wp.tile([128, 128], F16, name='norT')
            nc.vector.tensor_copy(norT[:], norT_ps)
            out_ps = pp.tile([128, 512], F32, name='kv')
            nc.tensor.matmul(out_ps[:, 0:128], norT[:], wsb['wn2'][:], start=True,
                             stop=True)
            out_sb = wp.tile([128, 128], F32, name='out_sb')
            nc.scalar.activation(out_sb[:], out_ps[:, 0:128], AF.Copy,
                                 scale=stn[:, 6:7])
            nc.sync.dma_start(out=out_d[w * 128:(w + 1) * 128, :], in_=out_sb[:])

        worder = sorted(range(W), key=lambda w: (T[w], w))
        loaded = [load_window(worder[j]) for j in (0, 1) if j < W]
        for i0 in range(0, W, 2):
            ws = loaded
            loaded = [load_window(worder[j]) for j in (i0 + 2, i0 + 3) if j < W]
            for m in range(max(s['NM'] for s in ws)):
                for s in ws:
                    if m < s['NM']:
                        emit_macro(s, m)
            for s in ws:
                emit_tail(s)
    return nc


def kernel(_trace=False, **inputs):
    import bass_rust
    in_maps, T, base_t = _preprocess(inputs)
    nc = _build(T, base_t)
    bass_rust.generate_event_semaphores(nc)
    res = bass_utils.run_bass_kernel_spmd(nc, in_maps, core_ids=list(range(NCORES)),
                                          trace=_trace)
    out = np.concatenate(
        [np.asarray(res.results[c]['out'])[:NPC] for c in range(NCORES)], axis=0)
    if _trace:
        return out.astype(np.float32), res
    return out.astype(np.float32)

